# revision 30
# baseline (speedup 1.0000x reference)
"""MoE router kernel for Trainium2 (8 NeuronCores, token-parallel).

Computation per token t:
    logits[t, :]  = x[t] @ gate_w.T + gate_b                  # [64]
    top8 vals/idx of softmax(logits) == top8 of logits (monotonic)
    weights       = exp(v - v.max()) / sum(exp(v - v.max()))  # renormalized top-8
    expert_mask[e, j, t] = (idx[t, j] == e)

Sharding: tokens (16384) split 8 ways; gate weights replicated.
Device-side layout trick: all tall outputs are produced token-minor
(transposed) so SBUF partitions carry experts / k-slots; the tiny host-side
transposes at gather time are free compared to HW time.

The gate matmul streams x^T tiles (hidden on partitions) against a stationary
gate_w^T; x is transposed on the host during the shard scatter, so the device
only ever does perfectly-contiguous DMA reads.
"""

import os
import numpy as np

N_TOKENS = 16384
HIDDEN = 4096
E = 64          # experts
TOPK = 8
NCORES = 8
TOK = N_TOKENS // NCORES   # tokens per core


_BUILT = {}


def build_nc(nb=(1024, 1024), mm_f32r=False, reps=1):
    """Build + compile the per-core Bass module.

    nb: int (equal token blocks) or tuple of block sizes summing to TOK.
    reps>1 wraps the whole body in a hardware loop (benchmarking only)."""
    blocks = tuple(TOK // nb for _ in range(nb)) if isinstance(nb, int) else tuple(nb)
    assert sum(blocks) == TOK and all(b % 1024 == 0 for b in blocks)
    key = (blocks, mm_f32r, reps)
    if key in _BUILT:
        return _BUILT[key]

    from contextlib import ExitStack
    import concourse.bacc as bacc
    import concourse.tile as tile
    import concourse.mybir as mybir

    f32 = mybir.dt.float32
    f32r = mybir.dt.float32r
    i32 = mybir.dt.int32
    u32 = mybir.dt.uint32
    AF = mybir.ActivationFunctionType
    ALU = mybir.AluOpType

    NK = HIDDEN // 128      # contraction chunks

    dtm = f32r if mm_f32r else f32  # matmul operand dtype

    nc = bacc.Bacc(
        "TRN2", target_bir_lowering=False, debug=False, enable_asserts=False
    )

    xin = nc.dram_tensor("x_t", [HIDDEN, TOK], dtm, kind="ExternalInput")
    gw = nc.dram_tensor("gw_t", [128, NK * E], dtm, kind="ExternalInput")
    gb = nc.dram_tensor("gb", [128, 1], f32, kind="ExternalInput")
    ident = nc.dram_tensor("ident", [128, 128], f32, kind="ExternalInput")
    oneh = nc.dram_tensor("oneh", [8, 4 * 128], f32r, kind="ExternalInput")
    iota = nc.dram_tensor("iota", [128, 1], f32, kind="ExternalInput")

    o_lt = nc.dram_tensor("o_logitsT", [E, TOK], f32, kind="ExternalOutput")
    o_wt = nc.dram_tensor("o_wT", [TOPK, TOK], f32, kind="ExternalOutput")
    o_it = nc.dram_tensor("o_idxT", [TOPK, TOK], f32r, kind="ExternalOutput")
    o_mask = nc.dram_tensor("o_mask", [E, TOPK * TOK], mybir.dt.int8, kind="ExternalOutput")

    with tile.TileContext(nc) as tc, ExitStack() as ctx:
        cpool = ctx.enter_context(tc.tile_pool(name="const", bufs=1))
        gw_sb = cpool.tile([128, NK * E], dtm)
        nc.sync.dma_start(gw_sb[:, 0:E], gw.ap()[:, 0:E])
        nc.sync.dma_start(gw_sb[:, E:4 * E], gw.ap()[:, E:4 * E])
        nc.sync.dma_start(gw_sb[:, 4 * E:], gw.ap()[:, 4 * E:])
        gb2_sb = cpool.tile([128, 1], f32)
        nc.sync.dma_start(gb2_sb[:, :], gb.ap())
        id_sb = cpool.tile([128, 128], f32)
        nc.sync.dma_start(id_sb[:, :], ident.ap())
        oh_sb = cpool.tile([8, 4 * 128], f32r)
        nc.sync.dma_start(oh_sb[:, :], oneh.ap())
        io_sb = cpool.tile([128, 1], f32)
        nc.sync.dma_start(io_sb[:, :], iota.ap())

        apool = ctx.enter_context(tc.tile_pool(name="accum", bufs=1))
        logitsT_sb = apool.tile([128, TOK // 2], f32)
        wT_sb = apool.tile([TOPK, TOK], f32)
        idxT_sb = apool.tile([TOPK, TOK], f32r)
        mask_sb = apool.tile([128, 4 * TOK], mybir.dt.int8)

        xpool = ctx.enter_context(tc.tile_pool(name="x", bufs=8))
        ppool = ctx.enter_context(tc.tile_pool(name="psL", bufs=4, space="PSUM"))
        pmix = ctx.enter_context(tc.tile_pool(name="pmix", bufs=4, space="PSUM"))
        ptr = pw8 = pbc = pmix
        spool = ctx.enter_context(tc.tile_pool(name="small", bufs=3))

        rep_ctx = tc.For_i(0, reps, 1) if reps > 1 else None
        if rep_ctx is not None:
            ctx.enter_context(rep_ctx)
        t0 = 0
        for b, TB in enumerate(blocks):
            t0 = sum(blocks[:b])
            NQ = TB // 512
            NTL = TB // 128
            NPAIR = (NQ + 1) // 2
            psums = [
                ppool.tile([128, 512], f32, tag="pL", name=f"pL{b}_{q}")
                for q in range(NPAIR)
            ]
            # two h-chunks per DMA (1 MB transfers) for better HBM efficiency
            for k2 in range(NK // 2):
                xt = xpool.tile([128, 2 * TB], dtm, tag="x")
                src = xin.ap()[k2 * 256:(k2 + 1) * 256, t0:t0 + TB].rearrange(
                    "(a p) t -> p a t", p=128
                )
                dst = xt[:, :].rearrange("p (a t) -> p a t", a=2)
                if b == 0 and k2 == 0:
                    for a in range(2):
                        nc.sync.dma_start(dst[:, a:a + 1, :], src[:, a:a + 1, :])
                else:
                    nc.sync.dma_start(dst, src)
                for a in range(2):
                    k = 2 * k2 + a
                    for q in range(NQ):
                        h = q % 2
                        nc.tensor.matmul(
                            psums[q // 2][h * E:(h + 1) * E, :],
                            gw_sb[:, k * E:(k + 1) * E],
                            xt[:, a * TB + q * 512: a * TB + (q + 1) * 512],
                            start=(k == 0),
                            stop=(k == NK - 1),
                            tile_position=(0, h * E),
                            skip_group_check=True,
                        )
            # psum -> sbuf with per-expert bias add; logitsT_sb keeps the
            # (parity-half, pair) layout: partition h*64+e, free (t0+pair*512+t)/2
            ht0 = t0 // 2
            for q in range(NQ):
                h, pair = q % 2, q // 2
                nc.scalar.activation(
                    logitsT_sb[h * E:(h + 1) * E,
                               ht0 + pair * 512: ht0 + (pair + 1) * 512],
                    psums[q // 2][h * E:(h + 1) * E, :],
                    AF.Identity,
                    bias=gb2_sb[h * E:(h + 1) * E, 0:1],
                    scale=1.0,
                )

            # per-128-token tile, two passes so PE's in-order stream never
            # stalls on a tile's DVE/ACT chain:
            #   pass A: transpose logits tiles + top8 + weights into wx tiles
            #   pass B: transpose all wx tiles back token-minor
            wxs = []
            for i in range(NTL):
                c0 = t0 + i * 128
                q = i // 4
                h, pair = q % 2, q // 2
                lofs = ht0 + pair * 512 + (i % 4) * 128
                pt = ptr.tile([128, E], f32, tag="pmix")
                nc.tensor.transpose(
                    pt[:, :],
                    logitsT_sb[h * E:(h + 1) * E, lofs:lofs + 128],
                    id_sb[h * E:(h + 1) * E, h * E:(h + 1) * E],
                )
                lt = spool.tile([128, E], f32, tag="lt")
                nc.scalar.copy(lt[:, :], pt[:, :])
                v8 = spool.tile([128, TOPK], f32, tag="v8")
                nc.vector.max(out=v8[:, :], in_=lt[:, :])
                ixu = spool.tile([128, TOPK], u32, tag="ixu")
                nc.vector.max_index(out=ixu[:, :], in_max=v8[:, :], in_values=lt[:, :])
                negm = spool.tile([128, 1], f32, tag="negm")
                nc.vector.tensor_scalar_mul(negm[:, :], v8[:, 0:1], -1.0)
                e8 = spool.tile([128, TOPK], f32, tag="e8")
                s8 = spool.tile([128, 1], f32, tag="s8")
                nc.scalar.activation(
                    e8[:, :], v8[:, :], AF.Exp, bias=negm[:, 0:1], scale=1.0,
                    accum_out=s8[:, 0:1],
                )
                wx = spool.tile([128, 2 * TOPK], f32, tag="wx", bufs=14,
                                name=f"wx{b}_{i}")
                r8 = spool.tile([128, 1], f32, tag="r8")
                nc.vector.reciprocal(r8[:, :], s8[:, :])
                nc.vector.tensor_scalar_mul(wx[:, 0:TOPK], e8[:, :], r8[:, 0:1])
                nc.vector.tensor_copy(wx[:, TOPK:2 * TOPK], ixu[:, :])
                wxs.append(wx)
            for i, wx in enumerate(wxs):
                c0 = t0 + i * 128
                pw = pw8.tile([TOPK, 128], f32, tag="pmix")
                nc.tensor.transpose(pw[:, :], wx[:, 0:TOPK], id_sb[:, :])
                nc.scalar.copy(wT_sb[:, c0:c0 + 128], pw[:, :])
                pi = pw8.tile([TOPK, 128], f32, tag="pmix")
                nc.tensor.transpose(pi[:, :], wx[:, TOPK:2 * TOPK], id_sb[:, :])
                nc.scalar.copy(idxT_sb[:, c0:c0 + 128], pi[:, :])

            # expert mask: one-hot matmul broadcasts idxT rows jj (-> psum
            # partitions 0:64) and jj+4 (-> 64:128); compare vs expert id.
            # mask_sb partition p=(jh*64+e), free=(jj*TOK+t)
            for jj in range(4):
                for q in range(NQ):
                    pb = pbc.tile([128, 512], f32, tag="pmix")
                    nc.tensor.matmul(
                        pb[:, :],
                        oh_sb[:, jj * 128:(jj + 1) * 128],
                        idxT_sb[:, t0 + q * 512: t0 + (q + 1) * 512],
                        start=True,
                        stop=True,
                    )
                    nc.vector.tensor_scalar(
                        mask_sb[:, jj * TOK + t0 + q * 512: jj * TOK + t0 + (q + 1) * 512],
                        pb[:, :],
                        io_sb[:, 0:1],
                        None,
                        ALU.is_equal,
                    )

            # block outputs
            o_lt_v = o_lt.ap()[:, t0:t0 + TB].rearrange(
                "e (pair two t) -> e pair two t", two=2, t=512
            )
            l_v = logitsT_sb[:, ht0:ht0 + TB // 2].rearrange(
                "p (pair t) -> p pair t", t=512
            )
            for h in range(2):
                nc.scalar.dma_start(
                    o_lt_v[:, :, h, :], l_v[h * E:(h + 1) * E, :, :]
                )
            nc.scalar.dma_start(o_wt.ap()[:, t0:t0 + TB], wT_sb[:, t0:t0 + TB])
            nc.scalar.dma_start(o_it.ap()[:, t0:t0 + TB], idxT_sb[:, t0:t0 + TB])
            o_mask_v = o_mask.ap().rearrange("e (j t) -> e j t", t=TOK)
            mask_v = mask_sb[:, :].rearrange("p (jj t) -> p jj t", t=TOK)
            for jh in range(2):
                nc.scalar.dma_start(
                    o_mask_v[:, jh * 4:(jh + 1) * 4, t0:t0 + TB],
                    mask_v[jh * 64:(jh + 1) * 64, :, t0:t0 + TB],
                )

    nc.compile()
    _BUILT[key] = nc
    return nc


def _host_inputs(x, gate_w, gate_b):
    NK = HIDDEN // 128
    gwT = np.ascontiguousarray(gate_w.T).astype(np.float32, copy=False)
    gwp = np.ascontiguousarray(
        gwT.reshape(NK, 128, E).transpose(1, 0, 2).reshape(128, NK * E)
    )
    gb2 = np.ascontiguousarray(
        np.tile(gate_b.astype(np.float32).reshape(E), 2).reshape(128, 1)
    )
    ident = np.eye(128, dtype=np.float32)
    oneh = np.zeros((8, 4 * 128), dtype=np.float32)
    for jj in range(4):
        oneh[jj, jj * 128:jj * 128 + E] = 1.0
        oneh[jj + 4, jj * 128 + E:(jj + 1) * 128] = 1.0
    iota = np.tile(np.arange(E, dtype=np.float32), 2).reshape(128, 1)
    in_maps = []
    for c in range(NCORES):
        xT = np.ascontiguousarray(x[c * TOK:(c + 1) * TOK, :].T)
        in_maps.append(
            dict(x_t=xT, gw_t=gwp, gb=gb2, ident=ident, oneh=oneh, iota=iota)
        )
    return in_maps


def run(x, gate_w, gate_b, nb=(1024, 1024), mm_f32r=False, trace=False):
    from concourse.bass_utils import run_bass_kernel_spmd

    nc = build_nc(nb, mm_f32r)
    in_maps = _host_inputs(x, gate_w, gate_b)
    res = run_bass_kernel_spmd(
        nc, in_maps, core_ids=list(range(NCORES)), trace=False
    )
    outs = res.results

    logits = np.concatenate([r["o_logitsT"].T for r in outs], axis=0)
    weights = np.concatenate([r["o_wT"].T for r in outs], axis=0)
    indices = np.rint(
        np.concatenate([r["o_idxT"].T for r in outs], axis=0)
    ).astype(np.int32)
    mask = np.concatenate(
        [r["o_mask"].reshape(E, TOPK, TOK) for r in outs], axis=2
    ).astype(np.int32)
    return (logits, weights, indices, mask), res


def kernel(x, gate_w, gate_b):
    x = np.asarray(x, dtype=np.float32)
    gate_w = np.asarray(gate_w, dtype=np.float32)
    gate_b = np.asarray(gate_b, dtype=np.float32)
    out, _ = run(x, gate_w, gate_b)
    return out


# revision 32
# speedup vs baseline: 1.0847x; 1.0847x over previous
"""MoE router kernel for Trainium2 (8 NeuronCores, token-parallel).

Computation per token t:
    logits[t, :]  = x[t] @ gate_w.T + gate_b                  # [64]
    top8 vals/idx of softmax(logits) == top8 of logits (monotonic)
    weights       = exp(v - v.max()) / sum(exp(v - v.max()))  # renormalized top-8
    expert_mask[e, j, t] = (idx[t, j] == e)

Sharding: tokens (16384) split 8 ways; gate weights replicated.
Device-side layout trick: all tall outputs are produced token-minor
(transposed) so SBUF partitions carry experts / k-slots; the tiny host-side
transposes at gather time are free compared to HW time.

The gate matmul streams x^T tiles (hidden on partitions) against a stationary
gate_w^T; x is transposed on the host during the shard scatter, so the device
only ever does perfectly-contiguous DMA reads.
"""

import os
import numpy as np

N_TOKENS = 16384
HIDDEN = 4096
E = 64          # experts
TOPK = 8
NCORES = 8
TOK = N_TOKENS // NCORES   # tokens per core


_BUILT = {}


def build_nc(nb=(1024, 1024), mm_f32r=False, reps=1):
    """Build + compile the per-core Bass module.

    nb: int (equal token blocks) or tuple of block sizes summing to TOK.
    reps>1 wraps the whole body in a hardware loop (benchmarking only)."""
    blocks = tuple(TOK // nb for _ in range(nb)) if isinstance(nb, int) else tuple(nb)
    assert sum(blocks) == TOK and all(b % 1024 == 0 for b in blocks)
    key = (blocks, mm_f32r, reps)
    if key in _BUILT:
        return _BUILT[key]

    from contextlib import ExitStack
    import concourse.bacc as bacc
    import concourse.tile as tile
    import concourse.mybir as mybir

    f32 = mybir.dt.float32
    f32r = mybir.dt.float32r
    i32 = mybir.dt.int32
    u32 = mybir.dt.uint32
    AF = mybir.ActivationFunctionType
    ALU = mybir.AluOpType

    NK = HIDDEN // 128      # contraction chunks

    dtm = f32r if mm_f32r else f32  # matmul operand dtype

    nc = bacc.Bacc(
        "TRN2", target_bir_lowering=False, debug=False, enable_asserts=False
    )

    xin = nc.dram_tensor("x_t", [HIDDEN, TOK], dtm, kind="ExternalInput")
    gw = nc.dram_tensor("gw_t", [128, NK * E], dtm, kind="ExternalInput")
    gb = nc.dram_tensor("gb", [128, 1], f32, kind="ExternalInput")
    ident = nc.dram_tensor("ident", [128, 128], f32, kind="ExternalInput")
    oneh = nc.dram_tensor("oneh", [8, 4 * 128], f32r, kind="ExternalInput")
    iota = nc.dram_tensor("iota", [128, 1], f32, kind="ExternalInput")

    o_lt = nc.dram_tensor("o_logitsT", [E, TOK], f32, kind="ExternalOutput")
    o_wt = nc.dram_tensor("o_wT", [TOPK, TOK], f32, kind="ExternalOutput")
    o_it = nc.dram_tensor("o_idxT", [TOPK, TOK], f32r, kind="ExternalOutput")
    o_mask = nc.dram_tensor("o_mask", [E, TOPK * TOK], mybir.dt.int8, kind="ExternalOutput")

    with tile.TileContext(nc) as tc, ExitStack() as ctx:
        cpool = ctx.enter_context(tc.tile_pool(name="const", bufs=1))
        gw_sb = cpool.tile([128, NK * E], dtm)
        nc.sync.dma_start(gw_sb[:, 0:E], gw.ap()[:, 0:E])
        nc.sync.dma_start(gw_sb[:, E:4 * E], gw.ap()[:, E:4 * E])
        nc.sync.dma_start(gw_sb[:, 4 * E:], gw.ap()[:, 4 * E:])
        gb2_sb = cpool.tile([128, 1], f32)
        nc.sync.dma_start(gb2_sb[:, :], gb.ap())
        id_sb = cpool.tile([128, 128], f32)
        nc.sync.dma_start(id_sb[:, :], ident.ap())
        oh_sb = cpool.tile([8, 4 * 128], f32r)
        nc.sync.dma_start(oh_sb[:, :], oneh.ap())
        io_sb = cpool.tile([128, 1], f32)
        nc.sync.dma_start(io_sb[:, :], iota.ap())

        apool = ctx.enter_context(tc.tile_pool(name="accum", bufs=1))
        logitsT_sb = apool.tile([128, TOK // 2], f32)
        wT_sb = apool.tile([TOPK, TOK], f32)
        idxT_sb = apool.tile([TOPK, TOK], f32r)
        mask_sb = apool.tile([128, 4 * TOK], mybir.dt.int8)

        xpool = ctx.enter_context(tc.tile_pool(name="x", bufs=16))
        ppool = ctx.enter_context(tc.tile_pool(name="psL", bufs=4, space="PSUM"))
        pmix = ctx.enter_context(tc.tile_pool(name="pmix", bufs=4, space="PSUM"))
        ptr = pw8 = pbc = pmix
        spool = ctx.enter_context(tc.tile_pool(name="small", bufs=3))

        rep_ctx = tc.For_i(0, reps, 1) if reps > 1 else None
        if rep_ctx is not None:
            ctx.enter_context(rep_ctx)
        t0 = 0
        for b, TB in enumerate(blocks):
            t0 = sum(blocks[:b])
            NQ = TB // 512
            NTL = TB // 128
            NPAIR = (NQ + 1) // 2
            psums = [
                ppool.tile([128, 512], f32, tag="pL", name=f"pL{b}_{q}")
                for q in range(NPAIR)
            ]
            for k in range(NK):
                xt = xpool.tile([128, TB], dtm, tag="x")
                src = xin.ap()[k * 128:(k + 1) * 128, t0:t0 + TB]
                if b == 0 and k == 0:
                    for q in range(NQ):
                        nc.sync.dma_start(
                            xt[:, q * 512:(q + 1) * 512], src[:, q * 512:(q + 1) * 512]
                        )
                else:
                    nc.sync.dma_start(xt[:, :], src)
                for q in range(NQ):
                    h = q % 2
                    nc.tensor.matmul(
                        psums[q // 2][h * E:(h + 1) * E, :],
                        gw_sb[:, k * E:(k + 1) * E],
                        xt[:, q * 512:(q + 1) * 512],
                        start=(k == 0),
                        stop=(k == NK - 1),
                        tile_position=(0, h * E),
                        skip_group_check=True,
                    )
            # psum -> sbuf with per-expert bias add; logitsT_sb keeps the
            # (parity-half, pair) layout: partition h*64+e, free (t0+pair*512+t)/2
            ht0 = t0 // 2
            for q in range(NQ):
                h, pair = q % 2, q // 2
                nc.scalar.activation(
                    logitsT_sb[h * E:(h + 1) * E,
                               ht0 + pair * 512: ht0 + (pair + 1) * 512],
                    psums[q // 2][h * E:(h + 1) * E, :],
                    AF.Identity,
                    bias=gb2_sb[h * E:(h + 1) * E, 0:1],
                    scale=1.0,
                )

            # per-128-token tile, two passes so PE's in-order stream never
            # stalls on a tile's DVE/ACT chain:
            #   pass A: transpose logits tiles + top8 + weights into wx tiles
            #   pass B: transpose all wx tiles back token-minor
            wxs = []
            for i in range(NTL):
                c0 = t0 + i * 128
                q = i // 4
                h, pair = q % 2, q // 2
                lofs = ht0 + pair * 512 + (i % 4) * 128
                pt = ptr.tile([128, E], f32, tag="pmix")
                nc.tensor.transpose(
                    pt[:, :],
                    logitsT_sb[h * E:(h + 1) * E, lofs:lofs + 128],
                    id_sb[h * E:(h + 1) * E, h * E:(h + 1) * E],
                )
                lt = spool.tile([128, E], f32, tag="lt")
                nc.scalar.copy(lt[:, :], pt[:, :])
                v8 = spool.tile([128, TOPK], f32, tag="v8")
                nc.vector.max(out=v8[:, :], in_=lt[:, :])
                ixu = spool.tile([128, TOPK], u32, tag="ixu")
                nc.vector.max_index(out=ixu[:, :], in_max=v8[:, :], in_values=lt[:, :])
                negm = spool.tile([128, 1], f32, tag="negm")
                nc.vector.tensor_scalar_mul(negm[:, :], v8[:, 0:1], -1.0)
                e8 = spool.tile([128, TOPK], f32, tag="e8")
                s8 = spool.tile([128, 1], f32, tag="s8")
                nc.scalar.activation(
                    e8[:, :], v8[:, :], AF.Exp, bias=negm[:, 0:1], scale=1.0,
                    accum_out=s8[:, 0:1],
                )
                wx = spool.tile([128, 2 * TOPK], f32, tag="wx", bufs=14,
                                name=f"wx{b}_{i}")
                r8 = spool.tile([128, 1], f32, tag="r8")
                nc.vector.reciprocal(r8[:, :], s8[:, :])
                nc.vector.tensor_scalar_mul(wx[:, 0:TOPK], e8[:, :], r8[:, 0:1])
                nc.vector.tensor_copy(wx[:, TOPK:2 * TOPK], ixu[:, :])
                wxs.append(wx)
            for i, wx in enumerate(wxs):
                c0 = t0 + i * 128
                pw = pw8.tile([TOPK, 128], f32, tag="pmix")
                nc.tensor.transpose(pw[:, :], wx[:, 0:TOPK], id_sb[:, :])
                nc.scalar.copy(wT_sb[:, c0:c0 + 128], pw[:, :])
                pi = pw8.tile([TOPK, 128], f32, tag="pmix")
                nc.tensor.transpose(pi[:, :], wx[:, TOPK:2 * TOPK], id_sb[:, :])
                nc.scalar.copy(idxT_sb[:, c0:c0 + 128], pi[:, :])

            # expert mask: one-hot matmul broadcasts idxT rows jj (-> psum
            # partitions 0:64) and jj+4 (-> 64:128); compare vs expert id.
            # mask_sb partition p=(jh*64+e), free=(jj*TOK+t)
            for jj in range(4):
                for q in range(NQ):
                    pb = pbc.tile([128, 512], f32, tag="pmix")
                    nc.tensor.matmul(
                        pb[:, :],
                        oh_sb[:, jj * 128:(jj + 1) * 128],
                        idxT_sb[:, t0 + q * 512: t0 + (q + 1) * 512],
                        start=True,
                        stop=True,
                    )
                    nc.vector.tensor_scalar(
                        mask_sb[:, jj * TOK + t0 + q * 512: jj * TOK + t0 + (q + 1) * 512],
                        pb[:, :],
                        io_sb[:, 0:1],
                        None,
                        ALU.is_equal,
                    )

            # block outputs
            o_lt_v = o_lt.ap()[:, t0:t0 + TB].rearrange(
                "e (pair two t) -> e pair two t", two=2, t=512
            )
            l_v = logitsT_sb[:, ht0:ht0 + TB // 2].rearrange(
                "p (pair t) -> p pair t", t=512
            )
            for h in range(2):
                nc.scalar.dma_start(
                    o_lt_v[:, :, h, :], l_v[h * E:(h + 1) * E, :, :]
                )
            nc.scalar.dma_start(o_wt.ap()[:, t0:t0 + TB], wT_sb[:, t0:t0 + TB])
            nc.scalar.dma_start(o_it.ap()[:, t0:t0 + TB], idxT_sb[:, t0:t0 + TB])
            o_mask_v = o_mask.ap().rearrange("e (j t) -> e j t", t=TOK)
            mask_v = mask_sb[:, :].rearrange("p (jj t) -> p jj t", t=TOK)
            for jh in range(2):
                nc.scalar.dma_start(
                    o_mask_v[:, jh * 4:(jh + 1) * 4, t0:t0 + TB],
                    mask_v[jh * 64:(jh + 1) * 64, :, t0:t0 + TB],
                )

    nc.compile()
    _BUILT[key] = nc
    return nc


def _host_inputs(x, gate_w, gate_b):
    NK = HIDDEN // 128
    gwT = np.ascontiguousarray(gate_w.T).astype(np.float32, copy=False)
    gwp = np.ascontiguousarray(
        gwT.reshape(NK, 128, E).transpose(1, 0, 2).reshape(128, NK * E)
    )
    gb2 = np.ascontiguousarray(
        np.tile(gate_b.astype(np.float32).reshape(E), 2).reshape(128, 1)
    )
    ident = np.eye(128, dtype=np.float32)
    oneh = np.zeros((8, 4 * 128), dtype=np.float32)
    for jj in range(4):
        oneh[jj, jj * 128:jj * 128 + E] = 1.0
        oneh[jj + 4, jj * 128 + E:(jj + 1) * 128] = 1.0
    iota = np.tile(np.arange(E, dtype=np.float32), 2).reshape(128, 1)
    in_maps = []
    for c in range(NCORES):
        xT = np.ascontiguousarray(x[c * TOK:(c + 1) * TOK, :].T)
        in_maps.append(
            dict(x_t=xT, gw_t=gwp, gb=gb2, ident=ident, oneh=oneh, iota=iota)
        )
    return in_maps


def run(x, gate_w, gate_b, nb=(1024, 1024), mm_f32r=False, trace=False):
    from concourse.bass_utils import run_bass_kernel_spmd

    nc = build_nc(nb, mm_f32r)
    in_maps = _host_inputs(x, gate_w, gate_b)
    res = run_bass_kernel_spmd(
        nc, in_maps, core_ids=list(range(NCORES)), trace=False
    )
    outs = res.results

    logits = np.concatenate([r["o_logitsT"].T for r in outs], axis=0)
    weights = np.concatenate([r["o_wT"].T for r in outs], axis=0)
    indices = np.rint(
        np.concatenate([r["o_idxT"].T for r in outs], axis=0)
    ).astype(np.int32)
    mask = np.concatenate(
        [r["o_mask"].reshape(E, TOPK, TOK) for r in outs], axis=2
    ).astype(np.int32)
    return (logits, weights, indices, mask), res


def kernel(x, gate_w, gate_b):
    x = np.asarray(x, dtype=np.float32)
    gate_w = np.asarray(gate_w, dtype=np.float32)
    gate_b = np.asarray(gate_b, dtype=np.float32)
    out, _ = run(x, gate_w, gate_b)
    return out
